# revision 1
# baseline (speedup 1.0000x reference)
"""Trainium2 Bass kernel for nn_DAMIC_88235808129614.

TextCNN (embed -> conv fs=3/4/5 -> relu -> maxpool) + 2-layer LSTM + sigmoid
head. Data-parallel over batch across 8 NeuronCores; the sequential timestep
loop runs locally per shard (no collectives). fp16 matmul datapath with fp32
accumulation; LSTM layer-1 recurrent weights streamed from HBM each step.

kernel(**inputs) takes the FULL unsharded inputs and returns [64, 50, 32] f32.
"""
import numpy as np

import concourse.bass as bass
import concourse.mybir as mybir
import concourse.tile as tile
from concourse.bass_utils import run_bass_kernel_spmd
from concourse.masks import make_identity




def _patched_drain_and_barrier(self, tick_clock, wait_clock):
    drain_inst = self.nc.sync.drain()
    wait_clock.add_sem_waits(
        drain_inst.ins, tile.ScopedClock({None: tick_clock.global_clock})
    )
    si = drain_inst.ins.sync_info
    waits = list(si.on_wait) if si and si.on_wait else []
    if len(waits) > 1:
        si.on_wait = waits[:1]
        for w in waits[1:]:
            nop = self.nc.sync.nop(nofuse=True, hint="split_drain_wait")
            nsi = nop.ins.sync_info
            if nsi is None:
                nop.ins.sync_info = mybir.SyncInfo(on_wait=[w], on_update=[])
            else:
                nsi.on_wait = [w]
    self.nc.all_engine_barrier()
    assert self.sems is not None
    popped = self.nc._tile_sem_poison_stack.pop()
    assert popped is self._sem_poison
    self.nc.clear_and_free_semaphores(list(self.sems.allocated().values()))
    self.nc.all_engine_barrier()


def split_multiwait(nc, max_waits=1):
    """This walrus build rejects instructions carrying more than one sync
    wait. Move extra waits onto same-engine NoOps inserted just before the
    instruction (same-engine program order preserves the semantics)."""
    n = 0
    uid = 0
    for f in nc.m.functions:
        for bb in f.blocks:
            il = bb.instructions
            new = []
            for inst in il:
                si = inst.sync_info
                waits = list(si.on_wait) if si and si.on_wait else []
                if len(waits) > max_waits:
                    for w in waits[:-max_waits]:
                        uid += 1
                        nop = mybir.InstNoOp(
                            name=f"I-wsplit-{uid}", ins=[], outs=[])
                        nop.engine = inst.engine
                        nop.sync_info = mybir.SyncInfo(
                            on_wait=[w], on_update=[])
                        new.append(nop)
                        n += 1
                    si.on_wait = waits[-max_waits:]
                new.append(inst)
            il[:] = new
    return n


def apply():
    tile.TileContext._drain_and_barrier = _patched_drain_and_barrier


import numpy as np

import concourse.bass as bass
import concourse.mybir as mybir
import concourse.tile as tile
from concourse.masks import make_identity

F32 = mybir.dt.float32
F32R = mybir.dt.float32r
F16 = mybir.dt.float16
I32 = mybir.dt.int32

B, T, L = 64, 50, 64
VOCAB, EMB = 30000, 300
NF = 256
FS = (3, 4, 5)
HID = 1024
OUT = 32
N_CORES = 8
B_LOC = B // N_CORES          # 8
UTT = B_LOC * T               # 400 utterances per core
GROUPS = T                    # one group = one timestep = 8 utterances
TOK_PER_GROUP = B_LOC * L     # 512
E_CHUNKS = [(0, 128), (128, 128), (256, 44)]
G4 = 4 * HID                  # 4096


def build_nc(phases="all"):
    nc = bass.Bass()
    # ---- DRAM parameters (per-core inputs) ----
    tok = nc.declare_dram_parameter("tok", [UTT * L], I32, isOutput=False)
    emb = nc.declare_dram_parameter("emb", [VOCAB, EMB], F32, isOutput=False)
    w3 = nc.declare_dram_parameter("w3", [3, EMB, NF], F16, isOutput=False)
    w4 = nc.declare_dram_parameter("w4", [4, EMB, NF], F16, isOutput=False)
    w5 = nc.declare_dram_parameter("w5", [5, EMB, NF], F16, isOutput=False)
    cbias = nc.declare_dram_parameter("cbias", [3 * NF], F32, isOutput=False)
    h2oA = nc.declare_dram_parameter("h2oA", [768, OUT], F16, isOutput=False)
    h2oB = nc.declare_dram_parameter("h2oB", [HID, OUT], F16, isOutput=False)
    h2ob = nc.declare_dram_parameter("h2ob", [OUT], F32, isOutput=False)
    wih0b = nc.declare_dram_parameter("wih0b", [OUT + 2, G4], F16, isOutput=False)
    whh0T = nc.declare_dram_parameter("whh0T", [HID, G4], F16, isOutput=False)
    wih1T = nc.declare_dram_parameter("wih1T", [HID, G4], F16, isOutput=False)
    gw1 = nc.declare_dram_parameter("gw1", [HID, G4], F16, isOutput=False)
    bseld = nc.declare_dram_parameter("bseld", [34, B_LOC], F16, isOutput=False)
    y = nc.declare_dram_parameter("y", [T, OUT, B_LOC], F32, isOutput=True)

    wdr = {3: w3, 4: w4, 5: w5}
    POS = {3: 62, 4: 61, 5: 60}
    # feats chunk index for (fs, ftile)
    CHUNK = {(3, 0): 0, (3, 1): 1, (4, 0): 2, (4, 1): 3, (5, 0): 4, (5, 1): 5}

    with tile.TileContext(nc) as tc:
        from contextlib import ExitStack

        with ExitStack() as root:
            const = root.enter_context(tc.tile_pool(name="const", bufs=1))
            persist = root.enter_context(tc.tile_pool(name="persist", bufs=1))

            # identity for PE transpose
            ident = const.tile([128, 128], F32)
            make_identity(nc, ident[:])
            ident16 = const.tile([8, 8], F16)
            nc.vector.tensor_copy(out=ident16[:], in_=ident[:8, :8])

            # resident LSTM weights + head
            wih0b_sb = persist.tile([OUT + 2, G4], F16, tag="wih0b")
            nc.sync.dma_start(out=wih0b_sb[:], in_=wih0b[:])
            whh0_sb = []
            for c in range(8):
                t_ = persist.tile([128, G4], F16, tag=f"whh0_{c}")
                nc.sync.dma_start(out=t_[:], in_=whh0T[c * 128:(c + 1) * 128, :])
                whh0_sb.append(t_)
            wih1_sb = []
            for c in range(8):
                t1_ = persist.tile([128, G4], F16, tag=f"wih1_{c}",
                                   name=f"wih1_{c}")
                nc.sync.dma_start(out=t1_[:], in_=wih1T[c * 128:(c + 1) * 128, :])
                wih1_sb.append(t1_)
            h2oB_sb = persist.tile([128, 8 * OUT], F16, tag="h2oB")
            for c in range(8):
                nc.sync.dma_start(
                    out=h2oB_sb[:, c * OUT:(c + 1) * OUT],
                    in_=h2oB[c * 128:(c + 1) * 128, :],
                )
            h2ob_sb = persist.tile([OUT, 1], F32, tag="h2ob")
            nc.sync.dma_start(out=h2ob_sb[:], in_=h2ob[:].rearrange("(a b) -> a b", b=1))

            # ftA (feats live only in the CNN stack)
            ftA_sb = persist.tile([OUT, UTT], F32, tag="ftA")

            # LSTM states
            prevT = persist.tile([OUT + 1, B_LOC], F16, tag="prevT")
            nc.vector.memset(prevT[:], 0.0)
            nc.vector.memset(prevT[OUT:OUT + 1, :], 1.0)
            h0T = persist.tile([128, 8 * B_LOC], F16, tag="h0T")
            h1T = persist.tile([128, 8 * B_LOC], F16, tag="h1T")
            nc.vector.memset(h0T[:], 0.0)
            nc.vector.memset(h1T[:], 0.0)
            c0 = persist.tile([B_LOC, HID], F32, tag="c0")
            c1 = persist.tile([B_LOC, HID], F32, tag="c1")
            nc.vector.memset(c0[:], 0.0)
            nc.vector.memset(c1[:], 0.0)
            bsel = persist.tile([34, B_LOC], F16, tag="bsel")
            nc.sync.dma_start(out=bsel[:], in_=bseld[:])

            # ---------------- CNN phase ----------------
            with ExitStack() as cnn:
                wpool = cnn.enter_context(tc.tile_pool(name="wconv", bufs=1))
                idxp = cnn.enter_context(tc.tile_pool(name="idx", bufs=1))
                gath = cnn.enter_context(tc.tile_pool(name="gath", bufs=6))
                xp = cnn.enter_context(tc.tile_pool(name="xp", bufs=2))
                relup = cnn.enter_context(tc.tile_pool(name="relup", bufs=3))
                ps_tr = cnn.enter_context(
                    tc.tile_pool(name="ps_tr", bufs=3, space="PSUM"))
                ps_conv = cnn.enter_context(
                    tc.tile_pool(name="ps_conv", bufs=4, space="PSUM"))

                # conv weights resident: per (fs, k, echunk) tile [ec, 256]
                wsb = {}
                for fs in FS:
                    for k in range(fs):
                        for ci, (e0, ec) in enumerate(E_CHUNKS):
                            t_ = wpool.tile([128, NF], F16, tag=f"w{fs}_{k}_{ci}")
                            nc.sync.dma_start(
                                out=t_[:ec, :], in_=wdr[fs][k, e0:e0 + ec, :])
                            wsb[(fs, k, ci)] = t_
                # conv biases [128, 6] (col = chunk)
                cb_sb = wpool.tile([128, 6], F32, tag="cb")
                for (fs, ft), ch in CHUNK.items():
                    off = {3: 0, 4: NF, 5: 2 * NF}[fs] + ft * 128
                    nc.sync.dma_start(
                        out=cb_sb[:, ch:ch + 1],
                        in_=cbias[off:off + 128].rearrange("(a b) -> a b", b=1))

                h2oA_sb = wpool.tile([128, 6 * OUT], F16, tag="h2oA")
                for c in range(6):
                    nc.sync.dma_start(
                        out=h2oA_sb[:, c * OUT:(c + 1) * OUT],
                        in_=h2oA[c * 128:(c + 1) * 128, :],
                    )
                feats_sb = wpool.tile([128, 6 * UTT], F16, tag="feats")
                # all token indices [128, 200]
                idx_sb = idxp.tile([128, UTT * L // 128], I32, tag="idx")
                nc.sync.dma_start(
                    out=idx_sb[:], in_=tok[:].rearrange("(g p) -> p g", p=128))

                for g in (range(GROUPS) if phases in ("all", "cnn", "decoupled") else []):
                    # gather + transpose -> x_sb chunks [128, 512]
                    xs = [xp.tile([128, TOK_PER_GROUP], F16, tag=f"x{ci}",
                                  name=f"x{ci}")
                          for ci in range(3)]
                    for i in range(4):
                        rows = gath.tile([128, EMB], F32, tag="rows")
                        nc.gpsimd.indirect_dma_start(
                            out=rows[:],
                            out_offset=None,
                            in_=emb[:],
                            in_offset=bass.IndirectOffsetOnAxis(
                                ap=idx_sb[:, g * 4 + i:g * 4 + i + 1], axis=0),
                        )
                        for ci, (e0, ec) in enumerate(E_CHUNKS):
                            pt = ps_tr.tile([128, 128], F32, tag="pt")
                            nc.tensor.transpose(
                                out=pt[:ec, :], in_=rows[:, e0:e0 + ec],
                                identity=ident[:])
                            nc.vector.tensor_copy(
                                out=xs[ci][:ec, i * 128:(i + 1) * 128],
                                in_=pt[:ec, :])
                    # conv matmuls
                    for fs in FS:
                        npos = POS[fs]
                        for ft in range(2):
                            pc = ps_conv.tile([128, 8 * 62], F32, tag="conv")
                            first = True
                            for k in range(fs):
                                for ci, (e0, ec) in enumerate(E_CHUNKS):
                                    rhs = (xs[ci][:ec]
                                           .rearrange("e (n l) -> e n l", l=L)
                                           [:, :, k:k + npos])
                                    nc.tensor.matmul(
                                        pc[:, :8 * npos],
                                        lhsT=wsb[(fs, k, ci)][:ec,
                                             ft * 128:(ft + 1) * 128],
                                        rhs=rhs,
                                        start=first,
                                        stop=(k == fs - 1 and ci == 2),
                                    )
                                    first = False
                            ch = CHUNK[(fs, ft)]
                            relu = relup.tile([128, 8 * 62], F32, tag="relu")
                            nc.scalar.activation(
                                relu[:, :8 * npos], pc[:, :8 * npos],
                                mybir.ActivationFunctionType.Relu,
                                bias=cb_sb[:, ch:ch + 1])
                            nc.vector.tensor_reduce(
                                out=feats_sb[:, ch * UTT + g * 8:
                                             ch * UTT + (g + 1) * 8],
                                in_=relu[:, :8 * npos].rearrange(
                                    "f (n p) -> f n p", n=8),
                                axis=mybir.AxisListType.X,
                                op=mybir.AluOpType.max,
                            )
                # ftA = h2oA.T @ feats
                if phases == "lstm":
                    nc.vector.memset(feats_sb[:], 0.0)
                if phases == "decoupled":
                    nc.vector.memset(ftA_sb[:], 0.0)
                pf = ps_conv.tile([128, 8 * 62], F32, tag="conv")
                for c in (range(6) if phases != "decoupled" else []):
                    nc.tensor.matmul(
                        pf[:OUT, :UTT],
                        lhsT=h2oA_sb[:, c * OUT:(c + 1) * OUT],
                        rhs=feats_sb[:, c * UTT:(c + 1) * UTT],
                        start=(c == 0), stop=(c == 5),
                    )
                if phases != "decoupled":
                    nc.vector.tensor_copy(out=ftA_sb[:], in_=pf[:OUT, :UTT])

            # ---------------- LSTM phase ----------------
            with ExitStack() as lst:
                gw1p = lst.enter_context(tc.tile_pool(name="gw1p", bufs=3))
                gsb = lst.enter_context(tc.tile_pool(name="gates", bufs=1))
                tmpp = lst.enter_context(tc.tile_pool(name="tmp", bufs=1))
                outp = lst.enter_context(tc.tile_pool(name="outp", bufs=2))
                ps_g = lst.enter_context(
                    tc.tile_pool(name="ps_g", bufs=4, space="PSUM"))
                ps_s = lst.enter_context(
                    tc.tile_pool(name="ps_s", bufs=2, space="PSUM"))

                ACTF = mybir.ActivationFunctionType
                gfun = [ACTF.Sigmoid, ACTF.Sigmoid, ACTF.Tanh, ACTF.Sigmoid]

                gw1_r = gw1[:].rearrange("(c p) n -> p c n", p=128)

                if phases == "cnn":
                    nc.vector.memset(ftA_sb[:], 0.0)
                for t in (range(T) if phases in ("all", "lstm", "decoupled") else []):
                    # ---- layer 0 gates ----
                    g0 = [gsb.tile([B_LOC, HID], F16, tag=f"g0_{q}", name=f"g0_{q}")
                          for q in range(4)]
                    for n in range(8):
                        ps = ps_g.tile([B_LOC, 512], F32, tag="psn")
                        ns = slice(n * 512, (n + 1) * 512)
                        nc.tensor.matmul(ps[:], lhsT=prevT[:],
                                         rhs=wih0b_sb[:OUT + 1, ns],
                                         start=True, stop=False)
                        for c in range(8):
                            nc.tensor.matmul(
                                ps[:], lhsT=h0T[:, c * 8:(c + 1) * 8],
                                rhs=whh0_sb[c][:, ns],
                                start=False, stop=(c == 7))
                        nc.scalar.activation(
                            g0[n // 2][:, (n % 2) * 512:(n % 2 + 1) * 512],
                            ps[:], gfun[n // 2])
                    # ---- layer 0 cell update ----
                    t1 = tmpp.tile([B_LOC, HID], F16, tag="t1")
                    t2 = tmpp.tile([B_LOC, HID], F16, tag="t2")
                    hrow = tmpp.tile([B_LOC, HID], F16, tag="hrow0")
                    nc.vector.tensor_mul(t1[:], g0[1][:], c0[:])
                    nc.vector.tensor_mul(t2[:], g0[0][:], g0[2][:])
                    nc.vector.tensor_add(c0[:], t1[:], t2[:])
                    nc.scalar.activation(t1[:], c0[:], ACTF.Tanh)
                    nc.vector.tensor_mul(hrow[:], g0[3][:], t1[:])
                    pt = ps_s.tile([128, 8 * B_LOC], F16, tag="ptr")
                    for c in range(8):
                        nc.tensor.transpose(
                            out=pt[:, c * 8:(c + 1) * 8],
                            in_=hrow[:, c * 128:(c + 1) * 128],
                            identity=ident16[:])
                    nc.vector.tensor_copy(out=h0T[:], in_=pt[:])

                    # ---- layer 1 gates (streamed weights) ----
                    g1 = [gsb.tile([B_LOC, HID], F16, tag=f"g1_{q}", name=f"g1_{q}")
                          for q in range(4)]
                    for n in range(8):
                        ns = slice(n * 512, (n + 1) * 512)
                        wt = gw1p.tile([128, 8, 512], F16, tag="gw1t")
                        nc.sync.dma_start(out=wt[:], in_=gw1_r[:, :, ns])
                        ps = ps_g.tile([B_LOC, 512], F32, tag="psn")
                        for c in range(8):
                            nc.tensor.matmul(
                                ps[:], lhsT=h0T[:, c * 8:(c + 1) * 8],
                                rhs=wih1_sb[c][:, ns],
                                start=(c == 0), stop=False)
                        for c in range(8):
                            nc.tensor.matmul(
                                ps[:], lhsT=h1T[:, c * 8:(c + 1) * 8],
                                rhs=wt[:, c, :], start=False, stop=False)
                        nc.tensor.matmul(ps[:], lhsT=bsel[32:34, :],
                                         rhs=wih0b_sb[32:34, ns],
                                         start=False, stop=True)
                        nc.scalar.activation(
                            g1[n // 2][:, (n % 2) * 512:(n % 2 + 1) * 512],
                            ps[:], gfun[n // 2])
                    # ---- layer 1 cell update ----
                    u1 = tmpp.tile([B_LOC, HID], F16, tag="u1")
                    u2 = tmpp.tile([B_LOC, HID], F16, tag="u2")
                    hrow1 = tmpp.tile([B_LOC, HID], F16, tag="hrow1")
                    nc.vector.tensor_mul(u1[:], g1[1][:], c1[:])
                    nc.vector.tensor_mul(u2[:], g1[0][:], g1[2][:])
                    nc.vector.tensor_add(c1[:], u1[:], u2[:])
                    nc.scalar.activation(u1[:], c1[:], ACTF.Tanh)
                    nc.vector.tensor_mul(hrow1[:], g1[3][:], u1[:])
                    pt1 = ps_s.tile([128, 8 * B_LOC], F16, tag="ptr")
                    for c in range(8):
                        nc.tensor.transpose(
                            out=pt1[:, c * 8:(c + 1) * 8],
                            in_=hrow1[:, c * 128:(c + 1) * 128],
                            identity=ident16[:])
                    nc.vector.tensor_copy(out=h1T[:], in_=pt1[:])

                    # ---- prediction head ----
                    pp = ps_s.tile([OUT, B_LOC], F32, tag="ppred")
                    for c in range(8):
                        nc.tensor.matmul(
                            pp[:], lhsT=h2oB_sb[:, c * OUT:(c + 1) * OUT],
                            rhs=h1T[:, c * 8:(c + 1) * 8],
                            start=(c == 0), stop=(c == 7))
                    pin = outp.tile([OUT, B_LOC], F32, tag="pin")
                    nc.vector.tensor_add(pin[:], pp[:],
                                         ftA_sb[:, t * 8:(t + 1) * 8])
                    pred = outp.tile([OUT, B_LOC], F32, tag="pred")
                    nc.scalar.activation(pred[:], pin[:], ACTF.Sigmoid,
                                         bias=h2ob_sb[:])
                    nc.vector.tensor_copy(out=prevT[:OUT, :], in_=pred[:])
                    nc.sync.dma_start(out=y[t], in_=pred[:])
    return nc


def prep_inputs(dialogue, embedding, cw3, cb3, cw4, cb4, cw5, cb5,
                wih0, whh0, b0, wih1, whh1, b1, h2o_w, h2o_b):
    """Host-side: shard + lay out per-core input maps."""
    f32 = np.float32
    f16 = np.float16
    dial = np.asarray(dialogue).astype(np.int32)
    emb = np.ascontiguousarray(np.asarray(embedding, f32))
    w3p = np.ascontiguousarray(np.asarray(cw3, f32).transpose(2, 1, 0).astype(f16))
    w4p = np.ascontiguousarray(np.asarray(cw4, f32).transpose(2, 1, 0).astype(f16))
    w5p = np.ascontiguousarray(np.asarray(cw5, f32).transpose(2, 1, 0).astype(f16))
    cb = np.concatenate([np.asarray(cb3, f32), np.asarray(cb4, f32),
                         np.asarray(cb5, f32)])
    h2oA = np.ascontiguousarray(np.asarray(h2o_w, f32)[:, :768].T.astype(f16))
    h2oB = np.ascontiguousarray(np.asarray(h2o_w, f32)[:, 768:].T.astype(f16))
    h2ob = np.asarray(h2o_b, f32)
    wih0b = np.ascontiguousarray(
        np.concatenate([np.asarray(wih0, f32).T,
                        np.asarray(b0, f32)[None, :],
                        np.asarray(b1, f32)[None, :]], 0).astype(f16))
    whh0T = np.ascontiguousarray(np.asarray(whh0, f32).T.astype(f16))
    wih1Tp = np.ascontiguousarray(np.asarray(wih1, f32).T.astype(f16))
    gw1 = np.ascontiguousarray(np.asarray(whh1, f32).T.astype(f16))

    bsel_np = np.zeros((34, B_LOC), f16)
    bsel_np[33, :] = 1.0
    in_maps = []
    for c in range(N_CORES):
        tok = np.ascontiguousarray(
            dial[c * B_LOC:(c + 1) * B_LOC].transpose(1, 0, 2).reshape(-1))
        in_maps.append({
            "tok": tok, "emb": emb, "w3": w3p, "w4": w4p, "w5": w5p,
            "cbias": cb, "h2oA": h2oA, "h2oB": h2oB, "h2ob": h2ob,
            "wih0b": wih0b, "whh0T": whh0T, "wih1T": wih1Tp, "gw1": gw1,
            "bseld": bsel_np,
        })
    return in_maps


def assemble_output(results):
    """results: list of 8 dicts with y [T, OUT, B_LOC] -> [B, T, OUT]."""
    outs = []
    for c in range(N_CORES):
        yc = results[c]["y"]                       # [50, 32, 8]
        outs.append(np.ascontiguousarray(yc.transpose(2, 0, 1)))
    return np.concatenate(outs, 0).astype(np.float32)


_CACHE = {}


def kernel(**inputs) -> np.ndarray:
    apply()  # tile workarounds (idempotent)
    if "nc" not in _CACHE:
        nc = build_nc()
        split_multiwait(nc)
        _CACHE["nc"] = nc
    nc = _CACHE["nc"]
    in_maps = prep_inputs(**inputs)
    res = run_bass_kernel_spmd(nc, in_maps, core_ids=list(range(N_CORES)))
    return assemble_output(res.results)

